# revision 28
# baseline (speedup 1.0000x reference)
"""AthenaSA sliding-window attention layer on 8 TRN2 NeuronCores.

Sharding: sequence-parallel. 8 cores = 2 batches x 4 sequence chunks of 1024
tokens. Each core recomputes k/v for a 512-token halo (zero-padded for the
first chunk), so there are NO collectives.

v2: fp8(e4m3) everywhere the numerics allow.
  - Q/K/V/O projections run as fp8 DoubleRow matmuls (2 k-tiles of 128
    packed per instruction, 0.5 PE cycles/row): host pre-quantizes emb and
    weights with power-of-2 scales (wq*512 incl 1/sqrt(dk), wk/wv/wo*64),
    compensated in the exp scale (2^-15), the rts v-scale (rb/8 via the
    1/1024 ones column) and the fused residual multiply (1/512).
  - probs are fp8: scores sigma~0.8 so exp(s-1.5) fits e4m3's 240 max
    (softmax shift cancels); denominators+PV then run as fp8 DoubleRow
    over chunk pairs.
  - RMS stats from the fp8 emb via DoubleRow ones-matmuls.
  - Q-projection is interleaved with attention per head pair so the ACT
    exp stream overlaps PE; rope raw-term multiplies read PSUM directly
    on DVE (no ACT copy); v-scale copies and prob masking run on Pool.
  - masks: blk0 keeps full 5-chunk masks (kpos>=0 for core 0), blk1 uses
    a shared 2-chunk (ch0/ch4 triangle) mask via a strided view.
"""
import math
import os
import sys

sys.path.insert(0, "/opt/trn_rl_repo")

import numpy as np
import ml_dtypes

import concourse.bass as bass
import concourse.bacc as bacc
import concourse.mybir as mybir
from concourse import tile
from concourse import bass_utils
from contextlib import ExitStack

BF16 = ml_dtypes.bfloat16
F8 = ml_dtypes.float8_e4m3

B, S, E = 2, 4096, 2048
H, HKV, DK, DV = 16, 4, 128, 128
WIN = 512
EPS = 1e-5
TOWN, TALL, HALO = 1024, 1536, 512
NE = E // 128            # 16 e-tiles
NEP = NE // 2            # 8 e-tile pairs (DoubleRow)
NB = 2                   # window blocks per core
NQC = 4                  # query tiles of 128 per block
NCH = 5                  # key chunks of 128 per 640-window
GQ = H // HKV

SQ_W, SK_W, SV_W, SO_W, SA = 512.0, 64.0, 64.0, 64.0, 8.0
EXP_SCALE = 1.0 / (SQ_W * SK_W)      # 2^-15
EXP_BIAS = -1.5
RES_SCALE = 1.0 / (SA * SO_W)        # 2^-9
RTS_ONES = 1.0 / (128.0 * SA)        # rts = rb/8

f32 = mybir.dt.float32
bf = mybir.dt.bfloat16
f8 = mybir.dt.float8e4
AF = mybir.ActivationFunctionType
DR = mybir.MatmulPerfMode.DoubleRow
ALU = mybir.AluOpType


def build(tc, d):
    nc = tc.nc

    with ExitStack() as stage_all:
        stage_all.enter_context(
            nc.allow_low_precision(reason="fp8/bf16 compute path by design"))
        const_pool = stage_all.enter_context(tc.tile_pool(name="const", bufs=1))
        ones8 = const_pool.tile([128, 2, 128], f8)
        nc.gpsimd.memset(ones8[:], 1.0)
        epsb = const_pool.tile([128, 1], f32)
        nc.gpsimd.memset(epsb[:], EPS)
        invc = const_pool.tile([128, 1], bf)
        nc.gpsimd.memset(invc[:], RTS_ONES)
        expb = const_pool.tile([128, 1], f32)
        nc.gpsimd.memset(expb[:], EXP_BIAS)

        # manually-scoped pools; LIFO open/close order
        wo_cm = tc.tile_pool(name="wo", bufs=1)
        wo_pool = wo_cm.__enter__()
        res_cm = tc.tile_pool(name="res", bufs=8)
        res_pool = res_cm.__enter__()
        acat_cm = tc.tile_pool(name="acat", bufs=H // 2)
        acat_pool = acat_cm.__enter__()
        kT_cm = tc.tile_pool(name="kT", bufs=HKV)
        kT_pool = kT_cm.__enter__()
        v_cm = tc.tile_pool(name="v", bufs=1)
        v_pool = v_cm.__enter__()
        emb_cm = tc.tile_pool(name="embf8", bufs=1)
        emb_pool = emb_cm.__enter__()
        wkv_cm = tc.tile_pool(name="wkv", bufs=2)
        wkv_pool = wkv_cm.__enter__()
        rb = const_pool.tile([128, TALL], bf)
        rts = const_pool.tile([128, 12], f32)
        cosq = const_pool.tile([128, TOWN], bf)
        sinq = const_pool.tile([128, TOWN], bf)

        emb8 = emb_pool.tile([128, NE, TALL], f8, name="emb8")
        wk8 = wkv_pool.tile([128, NEP, 2, HKV * DK], f8, name="wk8")
        wv8 = wkv_pool.tile([128, NEP, 2, HKV * DV], f8, name="wv8")
        # wo prefetch: DMA issued just before the main loop
        wo8 = wo_pool.tile([128, H // 2, 2, E], f8, name="wo8")

        # ---------------- Stage A: RMSNorm stats ----------------
        with ExitStack() as sa:
            sq_pool = sa.enter_context(tc.tile_pool(name="sq", bufs=1))
            r_pool = sa.enter_context(tc.tile_pool(name="rms", bufs=1))
            ssq_psum = sa.enter_context(
                tc.tile_pool(name="ssq_ps", bufs=1, space="PSUM"))

            sq = sq_pool.tile([128, NE, TALL], f8)
            ssq = ssq_psum.tile([128, TALL], f32)  # 3 banks
            for ec in range(4):
                nc.sync.dma_start(emb8[:, 4 * ec:4 * (ec + 1), :],
                                  d["emb8"][:, 4 * ec:4 * (ec + 1), :])
            nc.sync.dma_start(wv8[:], d["wv"][:])
            nc.sync.dma_start(wk8[:], d["wk"][:])
            for ep in range(NEP):
                nc.vector.tensor_mul(sq[:, 2 * ep:2 * ep + 2, :],
                                     emb8[:, 2 * ep:2 * ep + 2, :],
                                     emb8[:, 2 * ep:2 * ep + 2, :])
                for j in range(3):
                    nc.tensor.matmul(
                        ssq[:, j * 512:(j + 1) * 512], ones8[:],
                        sq[:, 2 * ep:2 * ep + 2, j * 512:(j + 1) * 512],
                        start=(ep == 0), stop=(ep == NEP - 1), perf_mode=DR)
            # rms = sqrt(ssq/E + eps); rb = 1/rms (all rows identical)
            s_sb = r_pool.tile([128, TALL], f32)
            nc.scalar.activation(s_sb[:], ssq[:], AF.Sqrt,
                                 bias=epsb[:], scale=1.0 / E)
            rb_f = r_pool.tile([128, TALL], f32)
            nc.vector.reciprocal_approx_fast(rb_f[:], s_sb[:])
            nc.vector.tensor_copy(rb[:], rb_f[:])            # cast -> bf16
            # q-side rope factors, ready before the main loop
            nc.sync.dma_start(cosq[:], d["cosqT"][:, :])
            nc.sync.dma_start(sinq[:], d["sinqT"][:, :])
            nc.vector.tensor_mul(cosq[:], cosq[:], rb[:, HALO:])
            nc.vector.tensor_mul(sinq[:], sinq[:], rb[:, HALO:])

        # ---------------- Stage B1: K^T + V projections ----------------
        kT = []     # per kv head: [128(dk), TALL] bf16, rope'd, carries SK_W
        vt = v_pool.tile([128, 12, HKV * DV], f8, name="vt")  # carries SA
        with ExitStack() as sb1:
            cs_pool = sb1.enter_context(tc.tile_pool(name="cosk", bufs=1))
            tmp_pool = sb1.enter_context(tc.tile_pool(name="ropetmp", bufs=3))

            cosk = cs_pool.tile([128, TALL], bf)
            sink = cs_pool.tile([128, TALL], bf)
            nc.sync.dma_start(cosk[:], d["coskT"][:, :])
            nc.sync.dma_start(sink[:], d["sinkT"][:, :])
            nc.vector.tensor_mul(cosk[:], cosk[:], rb[:])
            nc.vector.tensor_mul(sink[:], sink[:], rb[:])

            # K first: its rope only needs rb; rts (for the vt copies) is
            # computed between K and V so V's copies never stall on it.
            with tc.tile_pool(name="k_ps", bufs=2, space="PSUM") as kps_pool:
                for hk in range(HKV):
                    kps = kps_pool.tile([128, TALL], f32)  # 3 banks
                    for ep in range(NEP):
                        for j in range(3):
                            nc.tensor.matmul(
                                kps[:, j * 512:(j + 1) * 512],
                                wk8[:, ep, :, hk * DK:(hk + 1) * DK],
                                emb8[:, 2 * ep:2 * ep + 2,
                                     j * 512:(j + 1) * 512],
                                start=(ep == 0), stop=(ep == NEP - 1),
                                perf_mode=DR)
                    # rope: ko = cos*kraw + sin*swap(kraw); raw term reads
                    # PSUM directly on DVE; swap via 2 ACT copies
                    t1 = tmp_pool.tile([128, TALL], bf)
                    nc.vector.tensor_mul(t1[:], kps[:], cosk[:])
                    ksw = tmp_pool.tile([128, TALL], bf)
                    nc.scalar.copy(ksw[0:64, :], kps[64:128, :])
                    nc.scalar.copy(ksw[64:128, :], kps[0:64, :])
                    ko = kT_pool.tile([128, TALL], bf, name="ko")
                    nc.vector.tensor_mul(ko[:], ksw[:], sink[:])
                    nc.vector.tensor_add(ko[:], ko[:], t1[:])
                    kT.append(ko)
                # rts[t] = rb[token t]/8 via tiny matmuls (rows identical)
                rtp = kps_pool.tile([128, 12], f32)
                for t in range(12):
                    nc.tensor.matmul(rtp[:, t:t + 1],
                                     rb[:, t * 128:(t + 1) * 128],
                                     invc[:], start=True, stop=True)
                nc.vector.tensor_copy(rts[:], rtp[:])

            with tc.tile_pool(name="v_ps", bufs=5, space="PSUM") as vps_pool:
                for t in range(12):
                    vps = vps_pool.tile([128, HKV * DV], f32)  # 1 bank
                    for ep in range(NEP):
                        nc.tensor.matmul(
                            vps[:], emb8[:, 2 * ep:2 * ep + 2,
                                         t * 128:(t + 1) * 128],
                            wv8[:, ep], start=(ep == 0), stop=(ep == NEP - 1),
                            perf_mode=DR)
                    # vt = vps * rb/8  (ACT copy with per-token scale, fp8 out)
                    nc.scalar.activation(vt[:, t, :], vps[:], AF.Copy,
                                         scale=rts[:, t:t + 1])

        wkv_cm.__exit__(None, None, None)   # wk8/wv8 dead past B1

        # ------------- Main loop: Q proj + attention, per head pair -------
        acat = []
        for p in range(H // 2):
            acat.append(acat_pool.tile([128, 8 * 256], f8, name="acat"))

        with ExitStack() as sc_stage:
            wq_pool = sc_stage.enter_context(tc.tile_pool(name="wq", bufs=3))
            qT_pool = sc_stage.enter_context(tc.tile_pool(name="qT", bufs=3))
            tmpq_pool = sc_stage.enter_context(
                tc.tile_pool(name="ropetmpq", bufs=4))
            mask_pool = sc_stage.enter_context(tc.tile_pool(name="mask", bufs=2))
            probs_pool = sc_stage.enter_context(
                tc.tile_pool(name="probs", bufs=4))
            rec_pool = sc_stage.enter_context(tc.tile_pool(name="rec", bufs=3))
            qps_pool = sc_stage.enter_context(
                tc.tile_pool(name="q_ps", bufs=1, space="PSUM"))
            scps_pool = sc_stage.enter_context(
                tc.tile_pool(name="sc_ps", bufs=1, space="PSUM"))
            dno_pool = sc_stage.enter_context(
                tc.tile_pool(name="dno_ps", bufs=1, space="PSUM"))

            maskB = mask_pool.tile([128, 2, 256], f8, name="mB")
            nc.sync.dma_start(
                maskB[:], d["maskB"].rearrange("k c g q -> k c (g q)"))
            # dn stationary: per-iteration key-validity (zeroes padded keys)
            von = mask_pool.tile([128, NB * NQC, 2, 2, 128], f8, name="von")
            nc.sync.dma_start(von[:], d["vones"][:])
            # residual tiles prefetched while the loop runs (DMA is idle here)
            emb_res = []
            for t in range(8):
                et = res_pool.tile([128, E], bf, name="embres")
                nc.sync.dma_start(et[:], d["emb_own"][t * 128:(t + 1) * 128, :])
                emb_res.append(et)

            # scores psum: manual double buffer, 5 banks
            scpbig = scps_pool.tile([128, 2, NCH * 256], f32)

            def qproj_h(p, hh, qpair):
                h = 2 * p + hh
                wqh = wq_pool.tile([128, NEP, 2, DK], f8, name="wqh")
                nc.sync.dma_start(wqh[:], d["wq"][h])
                if hh == 1:  # spread the wo prefetch across the loop
                    nc.sync.dma_start(wo8[:, p], d["wo"][:, p])
                qps = qps_pool.tile([128, TOWN], f32)  # 2 banks
                for ep in range(NEP):
                    for j in range(2):
                        nc.tensor.matmul(
                            qps[:, j * 512:(j + 1) * 512],
                            wqh[:, ep],
                            emb8[:, 2 * ep:2 * ep + 2,
                                 HALO + j * 512:HALO + (j + 1) * 512],
                            start=(ep == 0), stop=(ep == NEP - 1),
                            perf_mode=DR)
                # rope: raw term from PSUM on DVE, swap via ACT
                t1 = tmpq_pool.tile([128, TOWN], bf, name="t1q")
                nc.vector.tensor_mul(t1[:], qps[:], cosq[:])
                qsw = tmpq_pool.tile([128, TOWN], bf)
                nc.scalar.copy(qsw[0:64, :], qps[64:128, :])
                nc.scalar.copy(qsw[64:128, :], qps[0:64, :])
                qo = qpair.rearrange("p (t g q) -> p t g q",
                                     g=2, q=128)[:, :, hh, :]
                nc.vector.tensor_mul(qo, qsw[:], sinq[:])
                nc.vector.tensor_add(qo, qo, t1[:])

            def scores_emit(qpair, kv, t, buf):
                w0 = 128 * t
                scp = scpbig[:, buf, :]
                for ch in range(NCH):
                    nc.tensor.matmul(
                        scp[ :, ch * 256:(ch + 1) * 256],
                        kT[kv][:, w0 + ch * 128:w0 + (ch + 1) * 128],
                        qpair[:, t * 256:(t + 1) * 256],
                        start=True, stop=True)
                probs = probs_pool.tile([128, NCH * 256], f8)
                nc.scalar.activation(probs[:], scp, AF.Exp,
                                     scale=EXP_SCALE, bias=expb[:])
                # window triangles on ch0/ch4 only (strided view)
                pv = probs[:].rearrange(
                    "k (c x) -> k c x", c=NCH)[:, 0:NCH:4, :]
                nc.vector.tensor_mul(pv, pv, maskB[:])
                return probs

            def finish_emit(p, kv, t, probs):
                pch = probs[:].rearrange("k (c x) -> k c x", c=NCH)
                dno = dno_pool.tile([128, 512], f32)   # 1 bank
                dn = dno[:, 0:256]
                otp = dno[:, 256:512]
                for c in range(2):
                    nc.tensor.matmul(
                        dn, von[:, t, c], pch[:, 2 * c:2 * c + 2, :],
                        start=(c == 0), stop=False, perf_mode=DR)
                nc.tensor.matmul(dn, ones8[:, 0, :],
                                 probs[:, 4 * 256:5 * 256],
                                 start=False, stop=True)
                for c in range(2):
                    tt = t + 2 * c
                    nc.tensor.matmul(
                        otp, vt[:, tt:tt + 2, kv * DV:(kv + 1) * DV],
                        pch[:, 2 * c:2 * c + 2, :],
                        start=(c == 0), stop=False, perf_mode=DR)
                nc.tensor.matmul(
                    otp, vt[:, t + 4, kv * DV:(kv + 1) * DV],
                    probs[:, 4 * 256:5 * 256],
                    start=False, stop=True)
                rec = rec_pool.tile([128, 256], f32)
                nc.vector.reciprocal_approx_fast(rec[:], dn)
                nc.vector.tensor_mul(
                    acat[p][:, t * 256:(t + 1) * 256], otp, rec[:])

            qpair_cur = qT_pool.tile([128, 2 * TOWN], bf, name="qpair")
            qproj_h(0, 0, qpair_cur)
            qproj_h(0, 1, qpair_cur)
            pending = None
            it = 0
            for p in range(H // 2):
                kv = p // 2
                qpair = qpair_cur
                if p + 1 < H // 2:
                    qpair_cur = qT_pool.tile([128, 2 * TOWN], bf, name="qpair")
                    qproj_h(p + 1, 0, qpair_cur)
                for t in range(8):
                    if t == 4 and p + 1 < H // 2:
                        qproj_h(p + 1, 1, qpair_cur)
                    probs = scores_emit(qpair, kv, t, it % 2)
                    if pending is not None:
                        finish_emit(*pending)
                    pending = (p, kv, t, probs)
                    it += 1
            finish_emit(*pending)
        emb_cm.__exit__(None, None, None)
        v_cm.__exit__(None, None, None)
        kT_cm.__exit__(None, None, None)

        # ---------------- Stage D: out projection + residual ----------
        with ExitStack() as sd:
            out_pool = sd.enter_context(tc.tile_pool(name="outsb", bufs=3))
            ops_pool = sd.enter_context(
                tc.tile_pool(name="op_ps", bufs=4, space="PSUM"))

            for t in range(8):
                out_sb = out_pool.tile([128, E], bf)
                for j in range(4):
                    op = ops_pool.tile([128, 512], f32)
                    for pr in range(H // 2):
                        lhs = acat[pr][:, t * 256:(t + 1) * 256].rearrange(
                            "p (g q) -> p g q", g=2)
                        nc.tensor.matmul(
                            op[:], lhs,
                            wo8[:, pr, :, j * 512:(j + 1) * 512],
                            start=(pr == 0), stop=(pr == H // 2 - 1),
                            perf_mode=DR)
                    nc.vector.scalar_tensor_tensor(
                        out_sb[:, j * 512:(j + 1) * 512], op[:], RES_SCALE,
                        emb_res[t][:, j * 512:(j + 1) * 512], ALU.mult,
                        ALU.add)
                nc.sync.dma_start(d["out"][t * 128:(t + 1) * 128, :],
                                  out_sb[:])
        acat_cm.__exit__(None, None, None)
        res_cm.__exit__(None, None, None)
        wo_cm.__exit__(None, None, None)


_CACHED_NC = None


def build_graph():
    global _CACHED_NC
    if _CACHED_NC is not None:
        return _CACHED_NC
    nc = bacc.Bacc("TRN2", target_bir_lowering=False, debug=False,
                   enable_asserts=False, num_devices=8)
    d = {}
    d["emb8"] = nc.dram_tensor("emb8", [128, NE, TALL], f8,
                               kind="ExternalInput").ap()
    d["emb_own"] = nc.dram_tensor("emb_own", [TOWN, E], bf,
                                  kind="ExternalInput").ap()
    d["wq"] = nc.dram_tensor("wq", [H, 128, NEP, 2, DK], f8,
                             kind="ExternalInput").ap()
    d["wk"] = nc.dram_tensor("wk", [128, NEP, 2, HKV * DK], f8,
                             kind="ExternalInput").ap()
    d["wv"] = nc.dram_tensor("wv", [128, NEP, 2, HKV * DV], f8,
                             kind="ExternalInput").ap()
    d["wo"] = nc.dram_tensor("wo", [128, H // 2, 2, E], f8,
                             kind="ExternalInput").ap()
    d["cosqT"] = nc.dram_tensor("cosqT", [DK, TOWN], bf, kind="ExternalInput").ap()
    d["sinqT"] = nc.dram_tensor("sinqT", [DK, TOWN], bf, kind="ExternalInput").ap()
    d["coskT"] = nc.dram_tensor("coskT", [DK, TALL], bf, kind="ExternalInput").ap()
    d["sinkT"] = nc.dram_tensor("sinkT", [DK, TALL], bf, kind="ExternalInput").ap()
    d["maskB"] = nc.dram_tensor("maskB", [128, 2, 2, 128], f8,
                                kind="ExternalInput").ap()
    d["vones"] = nc.dram_tensor("vones", [128, NB * NQC, 2, 2, 128], f8,
                                kind="ExternalInput").ap()
    d["out"] = nc.dram_tensor("out", [TOWN, E], bf, kind="ExternalOutput").ap()

    with tile.TileContext(nc, trace_sim=False) as tc:
        build(tc, d)
    nc.compile()
    _CACHED_NC = nc
    return nc


def make_in_maps(embeddings, cos_buffer, sin_buffer, wq, wk, wv, wo):
    embeddings = np.asarray(embeddings, dtype=np.float32)
    cos_buffer = np.asarray(cos_buffer, dtype=np.float32)
    sin_buffer = np.asarray(sin_buffer, dtype=np.float32)

    # fp8 weight packs with power-of-2 scales; DoubleRow pair layouts
    wq_s = (np.asarray(wq, np.float32) * (SQ_W / math.sqrt(DK))).astype(F8)
    # [E, H*DK] -> per head [E, DK] -> [NEP, 2, 128, DK] -> [128, NEP, 2, DK]
    wq_p = np.ascontiguousarray(
        wq_s.reshape(NEP, 2, 128, H, DK).transpose(3, 2, 0, 1, 4))
    wk_p = np.ascontiguousarray(
        (np.asarray(wk, np.float32) * SK_W).astype(F8)
        .reshape(NEP, 2, 128, HKV * DK).transpose(2, 0, 1, 3))
    wv_p = np.ascontiguousarray(
        (np.asarray(wv, np.float32) * SV_W).astype(F8)
        .reshape(NEP, 2, 128, HKV * DV).transpose(2, 0, 1, 3))
    wo_p = np.ascontiguousarray(
        (np.asarray(wo, np.float32) * SO_W).astype(F8)
        .reshape(H // 2, 2, 128, E).transpose(2, 0, 1, 3))

    qq = np.arange(128)
    kk = np.arange(128)
    in_maps = []
    for core in range(8):
        b, c = divmod(core, 4)
        tok0 = 1024 * c
        if c == 0:
            pad = np.zeros((HALO, E), np.float32)
            seg = np.concatenate([pad, embeddings[b, :TOWN]], axis=0)
            padc = np.zeros((HALO, DK), np.float32)
            ck = np.concatenate([padc, cos_buffer[1, 0, :TOWN]], axis=0)
            sk = np.concatenate([padc, sin_buffer[1, 0, :TOWN]], axis=0)
        else:
            seg = embeddings[b, tok0 - HALO:tok0 + TOWN]
            ck = cos_buffer[1, 0, tok0 - HALO:tok0 + TOWN]
            sk = sin_buffer[1, 0, tok0 - HALO:tok0 + TOWN]

        # emb8: [E, TALL] -> [NE, 128, TALL] -> [128, NE, TALL] fp8
        emb8 = np.ascontiguousarray(
            seg.T.astype(F8).reshape(NE, 128, TALL).transpose(1, 0, 2))

        # vones: dn stationary with padded keys zeroed
        # [t=blk*4+qc, pair c, 128(kk), i, 128(out)] -> [128, t, c, i, 128]
        von = np.zeros((NB * NQC, 2, 128, 2, 128), np.float32)
        for blk in range(NB):
            for qc in range(NQC):
                t = 4 * blk + qc
                for c in range(2):
                    for i in range(2):
                        ch = 2 * c + i
                        kpos = (tok0 - 512 + 512 * blk + 128 * qc
                                + 128 * ch + kk)
                        von[t, c, :, i, :] = (kpos >= 0)[:, None]
        vones = np.ascontiguousarray(von.transpose(2, 0, 1, 3, 4))
        # maskB: shared ch0/ch4 window triangles (kk vs qq offsets)
        maskB = np.zeros((128, 2, 2, 128), np.float32)
        mch0 = (kk[:, None] - 512) > (qq[None, :] - WIN)   # j>i strict upper
        mch4 = kk[:, None] <= qq[None, :]                  # lower incl diag
        for g in range(2):
            maskB[:, 0, g, :] = mch0
            maskB[:, 1, g, :] = mch4

        in_maps.append({
            "emb8": emb8,
            "emb_own": np.ascontiguousarray(
                embeddings[b, tok0:tok0 + TOWN]).astype(BF16),
            "wq": wq_p, "wk": wk_p, "wv": wv_p, "wo": wo_p,
            "cosqT": np.ascontiguousarray(
                cos_buffer[0, 0, tok0:tok0 + TOWN].T).astype(BF16),
            "sinqT": np.ascontiguousarray(
                sin_buffer[0, 0, tok0:tok0 + TOWN].T).astype(BF16),
            "coskT": np.ascontiguousarray(ck.T).astype(BF16),
            "sinkT": np.ascontiguousarray(sk.T).astype(BF16),
            "maskB": maskB.astype(F8),
            "vones": vones.astype(F8),
        })
    return in_maps


def _install_ntff_hook():
    """Recreate the missing antenv.axon_hooks registry so
    run_bass_kernel_spmd(trace=True) can capture an NTFF profile."""
    import types
    if "antenv.axon_hooks" not in sys.modules:
        m = types.ModuleType("antenv.axon_hooks")
        m._hook = None
        m.set_axon_ntff_profile_hook = lambda h: setattr(m, "_hook", h)
        m.get_axon_ntff_profile_hook = lambda: m._hook
        sys.modules["antenv.axon_hooks"] = m
        try:
            import antenv
            antenv.axon_hooks = m
        except ImportError:
            pass
    try:
        from trn_agent_boot.trn_boot import _ntff_profile_via_ctypes
        hook = _ntff_profile_via_ctypes("/opt/axon/libaxon_pjrt.so")
        sys.modules["antenv.axon_hooks"].set_axon_ntff_profile_hook(hook)
    except Exception as exc:  # degrade to no tracing
        print(f"ntff hook install failed: {exc}", file=sys.stderr)


def kernel(embeddings, cos_buffer, sin_buffer, wq, wk, wv, wo, window_size,
           trace=False):
    assert int(window_size) == WIN
    if trace:
        _install_ntff_hook()
    nc = build_graph()
    in_maps = make_in_maps(embeddings, cos_buffer, sin_buffer, wq, wk, wv, wo)
    res = bass_utils.run_bass_kernel_spmd(
        nc, in_maps, core_ids=list(range(8)), trace=trace)
    out = np.zeros((B, S, E), np.float32)
    for core in range(8):
        b, c = divmod(core, 4)
        out[b, 1024 * c:1024 * (c + 1)] = np.asarray(
            res.results[core]["out"]).astype(np.float32)
    if trace:
        kernel.last_exec_time_ns = res.exec_time_ns
    return out


kernel.last_exec_time_ns = None


# revision 32
# speedup vs baseline: 1.0318x; 1.0318x over previous
"""AthenaSA sliding-window attention layer on 8 TRN2 NeuronCores.

Sharding: sequence-parallel. 8 cores = 2 batches x 4 sequence chunks of 1024
tokens. Each core recomputes k/v for a 512-token halo (zero-padded for the
first chunk), so there are NO collectives.

v2: fp8(e4m3) everywhere the numerics allow.
  - Q/K/V/O projections run as fp8 DoubleRow matmuls (2 k-tiles of 128
    packed per instruction, 0.5 PE cycles/row): host pre-quantizes emb and
    weights with power-of-2 scales (wq*512 incl 1/sqrt(dk), wk/wv/wo*64),
    compensated in the exp scale (2^-15), the rts v-scale (rb/8 via the
    1/1024 ones column) and the fused residual multiply (1/512).
  - probs are fp8: scores sigma~0.8 so exp(s-1.5) fits e4m3's 240 max
    (softmax shift cancels); denominators+PV then run as fp8 DoubleRow
    over chunk pairs.
  - RMS stats from the fp8 emb via DoubleRow ones-matmuls.
  - Q-projection is interleaved with attention per head pair so the ACT
    exp stream overlaps PE; rope raw-term multiplies read PSUM directly
    on DVE (no ACT copy); v-scale copies and prob masking run on Pool.
  - masks: blk0 keeps full 5-chunk masks (kpos>=0 for core 0), blk1 uses
    a shared 2-chunk (ch0/ch4 triangle) mask via a strided view.
"""
import math
import os
import sys

sys.path.insert(0, "/opt/trn_rl_repo")

import numpy as np
import ml_dtypes

import concourse.bass as bass
import concourse.bacc as bacc
import concourse.mybir as mybir
from concourse import tile
from concourse import bass_utils
from contextlib import ExitStack

BF16 = ml_dtypes.bfloat16
F8 = ml_dtypes.float8_e4m3

B, S, E = 2, 4096, 2048
H, HKV, DK, DV = 16, 4, 128, 128
WIN = 512
EPS = 1e-5
TOWN, TALL, HALO = 1024, 1536, 512
NE = E // 128            # 16 e-tiles
NEP = NE // 2            # 8 e-tile pairs (DoubleRow)
NB = 2                   # window blocks per core
NQC = 4                  # query tiles of 128 per block
NCH = 5                  # key chunks of 128 per 640-window
GQ = H // HKV

SQ_W, SK_W, SV_W, SO_W, SA = 512.0, 64.0, 64.0, 64.0, 8.0
EXP_SCALE = 1.0 / (SQ_W * SK_W)      # 2^-15
EXP_BIAS = -1.5
RES_SCALE = 1.0 / (SA * SO_W)        # 2^-9
RTS_ONES = 1.0 / (128.0 * SA)        # rts = rb/8

f32 = mybir.dt.float32
bf = mybir.dt.bfloat16
f8 = mybir.dt.float8e4
AF = mybir.ActivationFunctionType
DR = mybir.MatmulPerfMode.DoubleRow
ALU = mybir.AluOpType


def build(tc, d):
    nc = tc.nc

    with ExitStack() as stage_all:
        stage_all.enter_context(
            nc.allow_low_precision(reason="fp8/bf16 compute path by design"))
        const_pool = stage_all.enter_context(tc.tile_pool(name="const", bufs=1))
        ones8 = const_pool.tile([128, 2, 128], f8)
        nc.gpsimd.memset(ones8[:], 1.0)
        epsb = const_pool.tile([128, 1], f32)
        nc.gpsimd.memset(epsb[:], EPS)
        invc = const_pool.tile([128, 1], bf)
        nc.gpsimd.memset(invc[:], RTS_ONES)
        expb = const_pool.tile([128, 1], f32)
        nc.gpsimd.memset(expb[:], EXP_BIAS)

        # manually-scoped pools; LIFO open/close order
        wo_cm = tc.tile_pool(name="wo", bufs=1)
        wo_pool = wo_cm.__enter__()
        res_cm = tc.tile_pool(name="res", bufs=8)
        res_pool = res_cm.__enter__()
        acat_cm = tc.tile_pool(name="acat", bufs=H // 2)
        acat_pool = acat_cm.__enter__()
        kT_cm = tc.tile_pool(name="kT", bufs=HKV)
        kT_pool = kT_cm.__enter__()
        v_cm = tc.tile_pool(name="v", bufs=1)
        v_pool = v_cm.__enter__()
        emb_cm = tc.tile_pool(name="embf8", bufs=1)
        emb_pool = emb_cm.__enter__()
        wkv_cm = tc.tile_pool(name="wkv", bufs=2)
        wkv_pool = wkv_cm.__enter__()
        rb = const_pool.tile([128, TALL], bf)
        rts = const_pool.tile([128, 12], f32)
        cosq = const_pool.tile([128, TOWN], bf)
        sinq = const_pool.tile([128, TOWN], bf)

        emb8 = emb_pool.tile([128, NE, TALL], f8, name="emb8")
        wk8 = wkv_pool.tile([128, NEP, 2, HKV * DK], f8, name="wk8")
        wv8 = wkv_pool.tile([128, NEP, 2, HKV * DV], f8, name="wv8")
        # wo prefetch: DMA issued just before the main loop
        wo8 = wo_pool.tile([128, H // 2, 2, E], f8, name="wo8")

        # ---------------- Stage A: RMSNorm stats ----------------
        with ExitStack() as sa:
            sq_pool = sa.enter_context(tc.tile_pool(name="sq", bufs=1))
            r_pool = sa.enter_context(tc.tile_pool(name="rms", bufs=1))
            ssq_psum = sa.enter_context(
                tc.tile_pool(name="ssq_ps", bufs=1, space="PSUM"))

            sq = sq_pool.tile([128, NE, TALL], f8)
            ssq = ssq_psum.tile([128, TALL], f32)  # 3 banks
            nc.sync.dma_start(emb8[:, 0:4, :], d["emb8"][:, 0:4, :])
            nc.sync.dma_start(wk8[:], d["wk"][:])
            for ec in range(1, 4):
                nc.sync.dma_start(emb8[:, 4 * ec:4 * (ec + 1), :],
                                  d["emb8"][:, 4 * ec:4 * (ec + 1), :])
            nc.sync.dma_start(wv8[:], d["wv"][:])
            for ep in range(NEP):
                nc.vector.tensor_mul(sq[:, 2 * ep:2 * ep + 2, :],
                                     emb8[:, 2 * ep:2 * ep + 2, :],
                                     emb8[:, 2 * ep:2 * ep + 2, :])
                for j in range(3):
                    nc.tensor.matmul(
                        ssq[:, j * 512:(j + 1) * 512], ones8[:],
                        sq[:, 2 * ep:2 * ep + 2, j * 512:(j + 1) * 512],
                        start=(ep == 0), stop=(ep == NEP - 1), perf_mode=DR)
            # rms = sqrt(ssq/E + eps); rb = 1/rms (all rows identical)
            s_sb = r_pool.tile([128, TALL], f32)
            nc.scalar.activation(s_sb[:], ssq[:], AF.Sqrt,
                                 bias=epsb[:], scale=1.0 / E)
            rb_f = r_pool.tile([128, TALL], f32)
            nc.vector.reciprocal_approx_fast(rb_f[:], s_sb[:])
            nc.vector.tensor_copy(rb[:], rb_f[:])            # cast -> bf16
            # q-side rope factors, ready before the main loop
            nc.sync.dma_start(cosq[:], d["cosqT"][:, :])
            nc.sync.dma_start(sinq[:], d["sinqT"][:, :])
            nc.vector.tensor_mul(cosq[:], cosq[:], rb[:, HALO:])
            nc.vector.tensor_mul(sinq[:], sinq[:], rb[:, HALO:])

        # ---------------- Stage B1: K^T + V projections ----------------
        kT = []     # per kv head: [128(dk), TALL] bf16, rope'd, carries SK_W
        vt = v_pool.tile([128, 12, HKV * DV], f8, name="vt")  # carries SA
        with ExitStack() as sb1:
            cs_pool = sb1.enter_context(tc.tile_pool(name="cosk", bufs=1))
            tmp_pool = sb1.enter_context(tc.tile_pool(name="ropetmp", bufs=3))

            cosk = cs_pool.tile([128, TALL], bf)
            sink = cs_pool.tile([128, TALL], bf)
            nc.sync.dma_start(cosk[:], d["coskT"][:, :])
            nc.sync.dma_start(sink[:], d["sinkT"][:, :])
            nc.vector.tensor_mul(cosk[:], cosk[:], rb[:])
            nc.vector.tensor_mul(sink[:], sink[:], rb[:])

            # K first: its rope only needs rb; rts (for the vt copies) is
            # computed between K and V so V's copies never stall on it.
            with tc.tile_pool(name="k_ps", bufs=2, space="PSUM") as kps_pool:
                for hk in range(HKV):
                    kps = kps_pool.tile([128, TALL], f32)  # 3 banks
                    for ep in range(NEP):
                        for j in range(3):
                            nc.tensor.matmul(
                                kps[:, j * 512:(j + 1) * 512],
                                wk8[:, ep, :, hk * DK:(hk + 1) * DK],
                                emb8[:, 2 * ep:2 * ep + 2,
                                     j * 512:(j + 1) * 512],
                                start=(ep == 0), stop=(ep == NEP - 1),
                                perf_mode=DR)
                    # rope: ko = cos*kraw + sin*swap(kraw); raw term reads
                    # PSUM directly on DVE; swap via 2 ACT copies
                    t1 = tmp_pool.tile([128, TALL], bf)
                    nc.vector.tensor_mul(t1[:], kps[:], cosk[:])
                    ksw = tmp_pool.tile([128, TALL], bf)
                    nc.scalar.copy(ksw[0:64, :], kps[64:128, :])
                    nc.scalar.copy(ksw[64:128, :], kps[0:64, :])
                    ko = kT_pool.tile([128, TALL], bf, name="ko")
                    nc.vector.tensor_mul(ko[:], ksw[:], sink[:])
                    nc.vector.tensor_add(ko[:], ko[:], t1[:])
                    kT.append(ko)
                # rts[t] = rb[token t]/8 via tiny matmuls (rows identical)
                rtp = kps_pool.tile([128, 12], f32)
                for t in range(12):
                    nc.tensor.matmul(rtp[:, t:t + 1],
                                     rb[:, t * 128:(t + 1) * 128],
                                     invc[:], start=True, stop=True)
                nc.vector.tensor_copy(rts[:], rtp[:])

            with tc.tile_pool(name="v_ps", bufs=5, space="PSUM") as vps_pool:
                for t in range(12):
                    vps = vps_pool.tile([128, HKV * DV], f32)  # 1 bank
                    for ep in range(NEP):
                        nc.tensor.matmul(
                            vps[:], emb8[:, 2 * ep:2 * ep + 2,
                                         t * 128:(t + 1) * 128],
                            wv8[:, ep], start=(ep == 0), stop=(ep == NEP - 1),
                            perf_mode=DR)
                    # vt = vps * rb/8  (ACT copy with per-token scale, fp8 out)
                    nc.scalar.activation(vt[:, t, :], vps[:], AF.Copy,
                                         scale=rts[:, t:t + 1])

        wkv_cm.__exit__(None, None, None)   # wk8/wv8 dead past B1

        # ------------- Main loop: Q proj + attention, per head pair -------
        acat = []
        for p in range(H // 2):
            acat.append(acat_pool.tile([128, 8 * 256], f8, name="acat"))

        with ExitStack() as sc_stage:
            wq_pool = sc_stage.enter_context(tc.tile_pool(name="wq", bufs=3))
            qT_pool = sc_stage.enter_context(tc.tile_pool(name="qT", bufs=3))
            tmpq_pool = sc_stage.enter_context(
                tc.tile_pool(name="ropetmpq", bufs=6))
            mask_pool = sc_stage.enter_context(tc.tile_pool(name="mask", bufs=2))
            probs_pool = sc_stage.enter_context(
                tc.tile_pool(name="probs", bufs=4))
            rec_pool = sc_stage.enter_context(tc.tile_pool(name="rec", bufs=3))
            qps_pool = sc_stage.enter_context(
                tc.tile_pool(name="q_ps", bufs=1, space="PSUM"))
            scps_pool = sc_stage.enter_context(
                tc.tile_pool(name="sc_ps", bufs=1, space="PSUM"))
            dno_pool = sc_stage.enter_context(
                tc.tile_pool(name="dno_ps", bufs=1, space="PSUM"))

            maskB = mask_pool.tile([128, 2, 256], f8, name="mB")
            nc.sync.dma_start(
                maskB[:], d["maskB"].rearrange("k c g q -> k c (g q)"))
            # dn stationary: per-iteration key-validity (zeroes padded keys)
            von = mask_pool.tile([128, NB * NQC, 2, 2, 128], f8, name="von")
            nc.sync.dma_start(von[:], d["vones"][:])
            # residual tiles prefetched while the loop runs (DMA is idle here)
            emb_res = []
            for t in range(8):
                et = res_pool.tile([128, E], bf, name="embres")
                nc.sync.dma_start(et[:], d["emb_own"][t * 128:(t + 1) * 128, :])
                emb_res.append(et)

            # scores psum: manual double buffer, 5 banks
            scpbig = scps_pool.tile([128, 2, NCH * 256], f32)

            def qproj_h(p, hh, qpair):
                h = 2 * p + hh
                wqh = wq_pool.tile([128, NEP, 2, DK], f8, name="wqh")
                nc.sync.dma_start(wqh[:], d["wq"][h])
                if hh == 1:  # spread the wo prefetch across the loop
                    nc.sync.dma_start(wo8[:, p], d["wo"][:, p])
                qps = qps_pool.tile([128, TOWN], f32)  # 2 banks
                for ep in range(NEP):
                    for j in range(2):
                        nc.tensor.matmul(
                            qps[:, j * 512:(j + 1) * 512],
                            wqh[:, ep],
                            emb8[:, 2 * ep:2 * ep + 2,
                                 HALO + j * 512:HALO + (j + 1) * 512],
                            start=(ep == 0), stop=(ep == NEP - 1),
                            perf_mode=DR)
                # rope: qps drained by ACT alone (3 back-to-back copies) so
                # the single psum buffer frees fast; muls run off SBUF
                qraw = tmpq_pool.tile([128, TOWN], bf, name="qraw")
                nc.scalar.copy(qraw[:], qps[:])
                qsw = tmpq_pool.tile([128, TOWN], bf)
                nc.scalar.copy(qsw[0:64, :], qps[64:128, :])
                nc.scalar.copy(qsw[64:128, :], qps[0:64, :])
                t1 = tmpq_pool.tile([128, TOWN], bf, name="t1q")
                nc.vector.tensor_mul(t1[:], qraw[:], cosq[:])
                qo = qpair.rearrange("p (t g q) -> p t g q",
                                     g=2, q=128)[:, :, hh, :]
                nc.vector.tensor_mul(qo, qsw[:], sinq[:])
                nc.vector.tensor_add(qo, qo, t1[:])

            def scores_emit(qpair, kv, t, buf):
                w0 = 128 * t
                scp = scpbig[:, buf, :]
                for ch in range(NCH):
                    nc.tensor.matmul(
                        scp[ :, ch * 256:(ch + 1) * 256],
                        kT[kv][:, w0 + ch * 128:w0 + (ch + 1) * 128],
                        qpair[:, t * 256:(t + 1) * 256],
                        start=True, stop=True)
                probs = probs_pool.tile([128, NCH * 256], f8)
                nc.scalar.activation(probs[:], scp, AF.Exp,
                                     scale=EXP_SCALE, bias=expb[:])
                # window triangles on ch0/ch4 only (strided view)
                pv = probs[:].rearrange(
                    "k (c x) -> k c x", c=NCH)[:, 0:NCH:4, :]
                nc.vector.tensor_mul(pv, pv, maskB[:])
                return probs

            def finish_emit(p, kv, t, probs):
                pch = probs[:].rearrange("k (c x) -> k c x", c=NCH)
                dno = dno_pool.tile([128, 512], f32)   # 1 bank
                dn = dno[:, 0:256]
                otp = dno[:, 256:512]
                for c in range(2):
                    nc.tensor.matmul(
                        dn, von[:, t, c], pch[:, 2 * c:2 * c + 2, :],
                        start=(c == 0), stop=False, perf_mode=DR)
                nc.tensor.matmul(dn, ones8[:, 0, :],
                                 probs[:, 4 * 256:5 * 256],
                                 start=False, stop=True)
                for c in range(2):
                    tt = t + 2 * c
                    nc.tensor.matmul(
                        otp, vt[:, tt:tt + 2, kv * DV:(kv + 1) * DV],
                        pch[:, 2 * c:2 * c + 2, :],
                        start=(c == 0), stop=False, perf_mode=DR)
                nc.tensor.matmul(
                    otp, vt[:, t + 4, kv * DV:(kv + 1) * DV],
                    probs[:, 4 * 256:5 * 256],
                    start=False, stop=True)
                rec = rec_pool.tile([128, 256], f32)
                nc.vector.reciprocal_approx_fast(rec[:], dn)
                nc.vector.tensor_mul(
                    acat[p][:, t * 256:(t + 1) * 256], otp, rec[:])

            qpair_cur = qT_pool.tile([128, 2 * TOWN], bf, name="qpair")
            qproj_h(0, 0, qpair_cur)
            qproj_h(0, 1, qpair_cur)
            pend = []
            it = 0
            for p in range(H // 2):
                kv = p // 2
                qpair = qpair_cur
                if p + 1 < H // 2:
                    qpair_cur = qT_pool.tile([128, 2 * TOWN], bf, name="qpair")
                    qproj_h(p + 1, 0, qpair_cur)
                for t in range(8):
                    if t == 4 and p + 1 < H // 2:
                        qproj_h(p + 1, 1, qpair_cur)
                    probs = scores_emit(qpair, kv, t, it % 2)
                    pend.append((p, kv, t, probs))
                    if len(pend) > 2:
                        finish_emit(*pend.pop(0))
                    it += 1
            while pend:
                finish_emit(*pend.pop(0))
        emb_cm.__exit__(None, None, None)
        v_cm.__exit__(None, None, None)
        kT_cm.__exit__(None, None, None)

        # ---------------- Stage D: out projection + residual ----------
        with ExitStack() as sd:
            out_pool = sd.enter_context(tc.tile_pool(name="outsb", bufs=3))
            ops_pool = sd.enter_context(
                tc.tile_pool(name="op_ps", bufs=4, space="PSUM"))

            for t in range(8):
                out_sb = out_pool.tile([128, E], bf)
                for j in range(4):
                    op = ops_pool.tile([128, 512], f32)
                    for pr in range(H // 2):
                        lhs = acat[pr][:, t * 256:(t + 1) * 256].rearrange(
                            "p (g q) -> p g q", g=2)
                        nc.tensor.matmul(
                            op[:], lhs,
                            wo8[:, pr, :, j * 512:(j + 1) * 512],
                            start=(pr == 0), stop=(pr == H // 2 - 1),
                            perf_mode=DR)
                    nc.vector.scalar_tensor_tensor(
                        out_sb[:, j * 512:(j + 1) * 512], op[:], RES_SCALE,
                        emb_res[t][:, j * 512:(j + 1) * 512], ALU.mult,
                        ALU.add)
                nc.sync.dma_start(d["out"][t * 128:(t + 1) * 128, :],
                                  out_sb[:])
        acat_cm.__exit__(None, None, None)
        res_cm.__exit__(None, None, None)
        wo_cm.__exit__(None, None, None)


_CACHED_NC = None


def build_graph():
    global _CACHED_NC
    if _CACHED_NC is not None:
        return _CACHED_NC
    nc = bacc.Bacc("TRN2", target_bir_lowering=False, debug=False,
                   enable_asserts=False, num_devices=8)
    d = {}
    d["emb8"] = nc.dram_tensor("emb8", [128, NE, TALL], f8,
                               kind="ExternalInput").ap()
    d["emb_own"] = nc.dram_tensor("emb_own", [TOWN, E], bf,
                                  kind="ExternalInput").ap()
    d["wq"] = nc.dram_tensor("wq", [H, 128, NEP, 2, DK], f8,
                             kind="ExternalInput").ap()
    d["wk"] = nc.dram_tensor("wk", [128, NEP, 2, HKV * DK], f8,
                             kind="ExternalInput").ap()
    d["wv"] = nc.dram_tensor("wv", [128, NEP, 2, HKV * DV], f8,
                             kind="ExternalInput").ap()
    d["wo"] = nc.dram_tensor("wo", [128, H // 2, 2, E], f8,
                             kind="ExternalInput").ap()
    d["cosqT"] = nc.dram_tensor("cosqT", [DK, TOWN], bf, kind="ExternalInput").ap()
    d["sinqT"] = nc.dram_tensor("sinqT", [DK, TOWN], bf, kind="ExternalInput").ap()
    d["coskT"] = nc.dram_tensor("coskT", [DK, TALL], bf, kind="ExternalInput").ap()
    d["sinkT"] = nc.dram_tensor("sinkT", [DK, TALL], bf, kind="ExternalInput").ap()
    d["maskB"] = nc.dram_tensor("maskB", [128, 2, 2, 128], f8,
                                kind="ExternalInput").ap()
    d["vones"] = nc.dram_tensor("vones", [128, NB * NQC, 2, 2, 128], f8,
                                kind="ExternalInput").ap()
    d["out"] = nc.dram_tensor("out", [TOWN, E], bf, kind="ExternalOutput").ap()

    with tile.TileContext(nc, trace_sim=False) as tc:
        build(tc, d)
    nc.compile()
    _CACHED_NC = nc
    return nc


def make_in_maps(embeddings, cos_buffer, sin_buffer, wq, wk, wv, wo):
    embeddings = np.asarray(embeddings, dtype=np.float32)
    cos_buffer = np.asarray(cos_buffer, dtype=np.float32)
    sin_buffer = np.asarray(sin_buffer, dtype=np.float32)

    # fp8 weight packs with power-of-2 scales; DoubleRow pair layouts
    wq_s = (np.asarray(wq, np.float32) * (SQ_W / math.sqrt(DK))).astype(F8)
    # [E, H*DK] -> per head [E, DK] -> [NEP, 2, 128, DK] -> [128, NEP, 2, DK]
    wq_p = np.ascontiguousarray(
        wq_s.reshape(NEP, 2, 128, H, DK).transpose(3, 2, 0, 1, 4))
    wk_p = np.ascontiguousarray(
        (np.asarray(wk, np.float32) * SK_W).astype(F8)
        .reshape(NEP, 2, 128, HKV * DK).transpose(2, 0, 1, 3))
    wv_p = np.ascontiguousarray(
        (np.asarray(wv, np.float32) * SV_W).astype(F8)
        .reshape(NEP, 2, 128, HKV * DV).transpose(2, 0, 1, 3))
    wo_p = np.ascontiguousarray(
        (np.asarray(wo, np.float32) * SO_W).astype(F8)
        .reshape(H // 2, 2, 128, E).transpose(2, 0, 1, 3))

    qq = np.arange(128)
    kk = np.arange(128)
    in_maps = []
    for core in range(8):
        b, c = divmod(core, 4)
        tok0 = 1024 * c
        if c == 0:
            pad = np.zeros((HALO, E), np.float32)
            seg = np.concatenate([pad, embeddings[b, :TOWN]], axis=0)
            padc = np.zeros((HALO, DK), np.float32)
            ck = np.concatenate([padc, cos_buffer[1, 0, :TOWN]], axis=0)
            sk = np.concatenate([padc, sin_buffer[1, 0, :TOWN]], axis=0)
        else:
            seg = embeddings[b, tok0 - HALO:tok0 + TOWN]
            ck = cos_buffer[1, 0, tok0 - HALO:tok0 + TOWN]
            sk = sin_buffer[1, 0, tok0 - HALO:tok0 + TOWN]

        # emb8: [E, TALL] -> [NE, 128, TALL] -> [128, NE, TALL] fp8
        emb8 = np.ascontiguousarray(
            seg.T.astype(F8).reshape(NE, 128, TALL).transpose(1, 0, 2))

        # vones: dn stationary with padded keys zeroed
        # [t=blk*4+qc, pair c, 128(kk), i, 128(out)] -> [128, t, c, i, 128]
        von = np.zeros((NB * NQC, 2, 128, 2, 128), np.float32)
        for blk in range(NB):
            for qc in range(NQC):
                t = 4 * blk + qc
                for c in range(2):
                    for i in range(2):
                        ch = 2 * c + i
                        kpos = (tok0 - 512 + 512 * blk + 128 * qc
                                + 128 * ch + kk)
                        von[t, c, :, i, :] = (kpos >= 0)[:, None]
        vones = np.ascontiguousarray(von.transpose(2, 0, 1, 3, 4))
        # maskB: shared ch0/ch4 window triangles (kk vs qq offsets)
        maskB = np.zeros((128, 2, 2, 128), np.float32)
        mch0 = (kk[:, None] - 512) > (qq[None, :] - WIN)   # j>i strict upper
        mch4 = kk[:, None] <= qq[None, :]                  # lower incl diag
        for g in range(2):
            maskB[:, 0, g, :] = mch0
            maskB[:, 1, g, :] = mch4

        in_maps.append({
            "emb8": emb8,
            "emb_own": np.ascontiguousarray(
                embeddings[b, tok0:tok0 + TOWN]).astype(BF16),
            "wq": wq_p, "wk": wk_p, "wv": wv_p, "wo": wo_p,
            "cosqT": np.ascontiguousarray(
                cos_buffer[0, 0, tok0:tok0 + TOWN].T).astype(BF16),
            "sinqT": np.ascontiguousarray(
                sin_buffer[0, 0, tok0:tok0 + TOWN].T).astype(BF16),
            "coskT": np.ascontiguousarray(ck.T).astype(BF16),
            "sinkT": np.ascontiguousarray(sk.T).astype(BF16),
            "maskB": maskB.astype(F8),
            "vones": vones.astype(F8),
        })
    return in_maps


def _install_ntff_hook():
    """Recreate the missing antenv.axon_hooks registry so
    run_bass_kernel_spmd(trace=True) can capture an NTFF profile."""
    import types
    if "antenv.axon_hooks" not in sys.modules:
        m = types.ModuleType("antenv.axon_hooks")
        m._hook = None
        m.set_axon_ntff_profile_hook = lambda h: setattr(m, "_hook", h)
        m.get_axon_ntff_profile_hook = lambda: m._hook
        sys.modules["antenv.axon_hooks"] = m
        try:
            import antenv
            antenv.axon_hooks = m
        except ImportError:
            pass
    try:
        from trn_agent_boot.trn_boot import _ntff_profile_via_ctypes
        hook = _ntff_profile_via_ctypes("/opt/axon/libaxon_pjrt.so")
        sys.modules["antenv.axon_hooks"].set_axon_ntff_profile_hook(hook)
    except Exception as exc:  # degrade to no tracing
        print(f"ntff hook install failed: {exc}", file=sys.stderr)


def kernel(embeddings, cos_buffer, sin_buffer, wq, wk, wv, wo, window_size,
           trace=False):
    assert int(window_size) == WIN
    if trace:
        _install_ntff_hook()
    nc = build_graph()
    in_maps = make_in_maps(embeddings, cos_buffer, sin_buffer, wq, wk, wv, wo)
    res = bass_utils.run_bass_kernel_spmd(
        nc, in_maps, core_ids=list(range(8)), trace=trace)
    out = np.zeros((B, S, E), np.float32)
    for core in range(8):
        b, c = divmod(core, 4)
        out[b, 1024 * c:1024 * (c + 1)] = np.asarray(
            res.results[core]["out"]).astype(np.float32)
    if trace:
        kernel.last_exec_time_ns = res.exec_time_ns
    return out


kernel.last_exec_time_ns = None


# revision 38
# speedup vs baseline: 1.0697x; 1.0367x over previous
"""AthenaSA sliding-window attention layer on 8 TRN2 NeuronCores.

Sharding: sequence-parallel. 8 cores = 2 batches x 4 sequence chunks of 1024
tokens. Each core recomputes k/v for a 512-token halo (zero-padded for the
first chunk), so there are NO collectives.

v2: fp8(e4m3) everywhere the numerics allow.
  - Q/K/V/O projections run as fp8 DoubleRow matmuls (2 k-tiles of 128
    packed per instruction, 0.5 PE cycles/row): host pre-quantizes emb and
    weights with power-of-2 scales (wq*512 incl 1/sqrt(dk), wk/wv/wo*64),
    compensated in the exp scale (2^-15), the rts v-scale (rb/8 via the
    1/1024 ones column) and the fused residual multiply (1/512).
  - probs are fp8: scores sigma~0.8 so exp(s-1.5) fits e4m3's 240 max
    (softmax shift cancels); denominators+PV then run as fp8 DoubleRow
    over chunk pairs.
  - RMS stats from the fp8 emb via DoubleRow ones-matmuls.
  - Q-projection is interleaved with attention per head pair so the ACT
    exp stream overlaps PE; rope raw-term multiplies read PSUM directly
    on DVE (no ACT copy); v-scale copies and prob masking run on Pool.
  - masks: blk0 keeps full 5-chunk masks (kpos>=0 for core 0), blk1 uses
    a shared 2-chunk (ch0/ch4 triangle) mask via a strided view.
"""
import math
import os
import sys

sys.path.insert(0, "/opt/trn_rl_repo")

import numpy as np
import ml_dtypes

import concourse.bass as bass
import concourse.bacc as bacc
import concourse.mybir as mybir
from concourse import tile
from concourse import bass_utils
from contextlib import ExitStack

BF16 = ml_dtypes.bfloat16
F8 = ml_dtypes.float8_e4m3

B, S, E = 2, 4096, 2048
H, HKV, DK, DV = 16, 4, 128, 128
WIN = 512
EPS = 1e-5
TOWN, TALL, HALO = 1024, 1536, 512
NE = E // 128            # 16 e-tiles
NEP = NE // 2            # 8 e-tile pairs (DoubleRow)
NB = 2                   # window blocks per core
NQC = 4                  # query tiles of 128 per block
NCH = 5                  # key chunks of 128 per 640-window
GQ = H // HKV

SQ_W, SK_W, SV_W, SO_W, SA = 512.0, 64.0, 64.0, 64.0, 8.0
EXP_SCALE = 1.0 / (SQ_W * SK_W)      # 2^-15
EXP_BIAS = -1.5
RES_SCALE = 1.0 / (SA * SO_W)        # 2^-9
RTS_ONES = 1.0 / (128.0 * SA)        # rts = rb/8

f32 = mybir.dt.float32
bf = mybir.dt.bfloat16
f8 = mybir.dt.float8e4
AF = mybir.ActivationFunctionType
DR = mybir.MatmulPerfMode.DoubleRow
ALU = mybir.AluOpType


def build(tc, d):
    nc = tc.nc

    with ExitStack() as stage_all:
        stage_all.enter_context(
            nc.allow_low_precision(reason="fp8/bf16 compute path by design"))
        const_pool = stage_all.enter_context(tc.tile_pool(name="const", bufs=1))
        ones8 = const_pool.tile([128, 2, 128], f8)
        nc.gpsimd.memset(ones8[:], 1.0)
        epsb = const_pool.tile([128, 1], f32)
        nc.gpsimd.memset(epsb[:], EPS)
        invc = const_pool.tile([128, 1], bf)
        nc.gpsimd.memset(invc[:], RTS_ONES)
        expb = const_pool.tile([128, 1], f32)
        nc.gpsimd.memset(expb[:], EXP_BIAS)

        # manually-scoped pools; LIFO open/close order
        wo_cm = tc.tile_pool(name="wo", bufs=1)
        wo_pool = wo_cm.__enter__()
        res_cm = tc.tile_pool(name="res", bufs=8)
        res_pool = res_cm.__enter__()
        acat_cm = tc.tile_pool(name="acat", bufs=H // 2)
        acat_pool = acat_cm.__enter__()
        kT_cm = tc.tile_pool(name="kT", bufs=HKV)
        kT_pool = kT_cm.__enter__()
        v_cm = tc.tile_pool(name="v", bufs=1)
        v_pool = v_cm.__enter__()
        emb_cm = tc.tile_pool(name="embf8", bufs=1)
        emb_pool = emb_cm.__enter__()
        wkv_cm = tc.tile_pool(name="wkv", bufs=2)
        wkv_pool = wkv_cm.__enter__()
        rb = const_pool.tile([128, TALL], bf)
        rts = const_pool.tile([128, 12], f32)
        cosq = const_pool.tile([128, TOWN], bf)
        sinq = const_pool.tile([128, TOWN], bf)

        emb8 = emb_pool.tile([128, NE, TALL], f8, name="emb8")
        wk8 = wkv_pool.tile([128, NEP, 2, HKV * DK], f8, name="wk8")
        wv8 = wkv_pool.tile([128, NEP, 2, HKV * DV], f8, name="wv8")
        # wo prefetch: DMA issued just before the main loop
        wo8 = wo_pool.tile([128, H // 2, 2, E], f8, name="wo8")

        # ---------------- Stage A: RMSNorm stats ----------------
        with ExitStack() as sa:
            sq_pool = sa.enter_context(tc.tile_pool(name="sq", bufs=1))
            r_pool = sa.enter_context(tc.tile_pool(name="rms", bufs=1))
            ssq_psum = sa.enter_context(
                tc.tile_pool(name="ssq_ps", bufs=1, space="PSUM"))

            sq = sq_pool.tile([128, NE, TALL], f8)
            ssq = ssq_psum.tile([128, TALL], f32)  # 3 banks
            nc.sync.dma_start(emb8[:, 0:4, :], d["emb8"][:, 0:4, :])
            nc.sync.dma_start(wk8[:], d["wk"][:])
            for ec in range(1, 4):
                nc.sync.dma_start(emb8[:, 4 * ec:4 * (ec + 1), :],
                                  d["emb8"][:, 4 * ec:4 * (ec + 1), :])
            nc.sync.dma_start(wv8[:], d["wv"][:])
            for ep in range(NEP):
                nc.vector.tensor_mul(sq[:, 2 * ep:2 * ep + 2, :],
                                     emb8[:, 2 * ep:2 * ep + 2, :],
                                     emb8[:, 2 * ep:2 * ep + 2, :])
                for j in range(3):
                    nc.tensor.matmul(
                        ssq[:, j * 512:(j + 1) * 512], ones8[:],
                        sq[:, 2 * ep:2 * ep + 2, j * 512:(j + 1) * 512],
                        start=(ep == 0), stop=(ep == NEP - 1), perf_mode=DR)
            # rms = sqrt(ssq/E + eps); rb = 1/rms (all rows identical)
            s_sb = r_pool.tile([128, TALL], f32)
            nc.scalar.activation(s_sb[:], ssq[:], AF.Sqrt,
                                 bias=epsb[:], scale=1.0 / E)
            rb_f = r_pool.tile([128, TALL], f32)
            nc.vector.reciprocal_approx_fast(rb_f[:], s_sb[:])
            nc.vector.tensor_copy(rb[:], rb_f[:])            # cast -> bf16
            # q-side rope factors, ready before the main loop
            nc.sync.dma_start(cosq[:], d["cosqT"][:, :])
            nc.sync.dma_start(sinq[:], d["sinqT"][:, :])
            nc.vector.tensor_mul(cosq[:], cosq[:], rb[:, HALO:])
            nc.vector.tensor_mul(sinq[:], sinq[:], rb[:, HALO:])

        # ---------------- Stage B1: K^T + V projections ----------------
        kT = []     # per kv head: [128(dk), TALL] bf16, rope'd, carries SK_W
        vt = v_pool.tile([128, 12, HKV * DV], f8, name="vt")  # carries SA
        with ExitStack() as sb1:
            cs_pool = sb1.enter_context(tc.tile_pool(name="cosk", bufs=1))
            tmp_pool = sb1.enter_context(tc.tile_pool(name="ropetmp", bufs=3))

            cosk = cs_pool.tile([128, TALL], bf)
            sink = cs_pool.tile([128, TALL], bf)
            nc.sync.dma_start(cosk[:], d["coskT"][:, :])
            nc.sync.dma_start(sink[:], d["sinkT"][:, :])
            nc.vector.tensor_mul(cosk[:], cosk[:], rb[:])
            nc.vector.tensor_mul(sink[:], sink[:], rb[:])

            # K first: its rope only needs rb; rts (for the vt copies) is
            # computed between K and V so V's copies never stall on it.
            with tc.tile_pool(name="k_ps", bufs=2, space="PSUM") as kps_pool:
                for hk in range(HKV):
                    kps = kps_pool.tile([128, TALL], f32)  # 3 banks
                    for ep in range(NEP):
                        for j in range(3):
                            nc.tensor.matmul(
                                kps[:, j * 512:(j + 1) * 512],
                                wk8[:, ep, :, hk * DK:(hk + 1) * DK],
                                emb8[:, 2 * ep:2 * ep + 2,
                                     j * 512:(j + 1) * 512],
                                start=(ep == 0), stop=(ep == NEP - 1),
                                perf_mode=DR)
                    # rope: ko = cos*kraw + sin*swap(kraw); raw term reads
                    # PSUM directly on DVE; swap via 2 ACT copies
                    t1 = tmp_pool.tile([128, TALL], bf)
                    nc.vector.tensor_mul(t1[:], kps[:], cosk[:])
                    ksw = tmp_pool.tile([128, TALL], bf)
                    nc.scalar.copy(ksw[0:64, :], kps[64:128, :])
                    nc.scalar.copy(ksw[64:128, :], kps[0:64, :])
                    ko = kT_pool.tile([128, TALL], bf, name="ko")
                    nc.vector.tensor_mul(ko[:], ksw[:], sink[:])
                    nc.vector.tensor_add(ko[:], ko[:], t1[:])
                    kT.append(ko)
                # rts[t] = rb[token t]/8 via tiny matmuls (rows identical)
                rtp = kps_pool.tile([128, 12], f32)
                for t in range(12):
                    nc.tensor.matmul(rtp[:, t:t + 1],
                                     rb[:, t * 128:(t + 1) * 128],
                                     invc[:], start=True, stop=True)
                nc.vector.tensor_copy(rts[:], rtp[:])

            with tc.tile_pool(name="v_ps", bufs=5, space="PSUM") as vps_pool:
                for t in range(12):
                    vps = vps_pool.tile([128, HKV * DV], f32)  # 1 bank
                    for ep in range(NEP):
                        nc.tensor.matmul(
                            vps[:], emb8[:, 2 * ep:2 * ep + 2,
                                         t * 128:(t + 1) * 128],
                            wv8[:, ep], start=(ep == 0), stop=(ep == NEP - 1),
                            perf_mode=DR)
                    # vt = vps * rb/8  (ACT copy with per-token scale, fp8 out)
                    nc.scalar.activation(vt[:, t, :], vps[:], AF.Copy,
                                         scale=rts[:, t:t + 1])

        wkv_cm.__exit__(None, None, None)   # wk8/wv8 dead past B1

        # ------------- Main loop: Q proj + attention, per head pair -------
        acat = []
        for p in range(H // 2):
            acat.append(acat_pool.tile([128, 8 * 256], f8, name="acat"))

        with ExitStack() as sc_stage:
            wq_pool = sc_stage.enter_context(tc.tile_pool(name="wq", bufs=3))
            qT_pool = sc_stage.enter_context(tc.tile_pool(name="qT", bufs=3))
            tmpq_pool = sc_stage.enter_context(
                tc.tile_pool(name="ropetmpq", bufs=6))
            mask_pool = sc_stage.enter_context(tc.tile_pool(name="mask", bufs=2))
            probs_pool = sc_stage.enter_context(
                tc.tile_pool(name="probs", bufs=4))
            rec_pool = sc_stage.enter_context(tc.tile_pool(name="rec", bufs=3))
            qps_pool = sc_stage.enter_context(
                tc.tile_pool(name="q_ps", bufs=2, space="PSUM"))
            scps_pool = sc_stage.enter_context(
                tc.tile_pool(name="sc_ps", bufs=1, space="PSUM"))
            dno_pool = sc_stage.enter_context(
                tc.tile_pool(name="dno_ps", bufs=1, space="PSUM"))

            maskB = mask_pool.tile([128, 2, 256], f8, name="mB")
            nc.sync.dma_start(
                maskB[:], d["maskB"].rearrange("k c g q -> k c (g q)"))
            # dn stationary: per-iteration key-validity (zeroes padded keys)
            von = mask_pool.tile([128, NB * NQC, 2, 2, 128], f8, name="von")
            nc.sync.dma_start(von[:], d["vones"][:])
            # residual tiles: DMAs spread one-per-pair inside the loop
            emb_res = [res_pool.tile([128, E], bf, name="embres")
                       for _ in range(8)]

            def qproj_h(p, hh, qpair):
                h = 2 * p + hh
                wqh = wq_pool.tile([128, NEP, 2, DK], f8, name="wqh")
                nc.sync.dma_start(wqh[:], d["wq"][h])
                if hh == 1:  # spread the wo prefetch across the loop
                    nc.sync.dma_start(wo8[:, p], d["wo"][:, p])
                qps = qps_pool.tile([128, TOWN], f32)  # 2 banks
                for ep in range(NEP):
                    for j in range(2):
                        nc.tensor.matmul(
                            qps[:, j * 512:(j + 1) * 512],
                            wqh[:, ep],
                            emb8[:, 2 * ep:2 * ep + 2,
                                 HALO + j * 512:HALO + (j + 1) * 512],
                            start=(ep == 0), stop=(ep == NEP - 1),
                            perf_mode=DR)
                # rope: raw term from PSUM on DVE; swap via 2 ACT copies
                t1 = tmpq_pool.tile([128, TOWN], bf, name="t1q")
                nc.vector.tensor_mul(t1[:], qps[:], cosq[:])
                qsw = tmpq_pool.tile([128, TOWN], bf)
                nc.scalar.copy(qsw[0:64, :], qps[64:128, :])
                nc.scalar.copy(qsw[64:128, :], qps[0:64, :])
                qo = qpair.rearrange("p (t g q) -> p t g q",
                                     g=2, q=128)[:, :, hh, :]
                nc.vector.tensor_mul(qo, qsw[:], sinq[:])
                nc.vector.tensor_add(qo, qo, t1[:])

            def scores_emit(qpair, kv, t, buf):
                w0 = 128 * t
                scpt = scps_pool.tile([128, NCH * 256], f32, name="scpt")
                scp = scpt[:]  # 3 banks
                for ch in range(NCH):
                    nc.tensor.matmul(
                        scp[ :, ch * 256:(ch + 1) * 256],
                        kT[kv][:, w0 + ch * 128:w0 + (ch + 1) * 128],
                        qpair[:, t * 256:(t + 1) * 256],
                        start=True, stop=True)
                probs = probs_pool.tile([128, NCH * 256], f8)
                nc.scalar.activation(probs[:], scp, AF.Exp,
                                     scale=EXP_SCALE, bias=expb[:])
                # window triangles on ch0/ch4 only (strided view)
                pv = probs[:].rearrange(
                    "k (c x) -> k c x", c=NCH)[:, 0:NCH:4, :]
                nc.vector.tensor_mul(pv, pv, maskB[:])
                return probs

            def finish_emit(p, kv, t, probs):
                pch = probs[:].rearrange("k (c x) -> k c x", c=NCH)
                dno = dno_pool.tile([128, 512], f32)   # 1 bank
                dn = dno[:, 0:256]
                otp = dno[:, 256:512]
                for c in range(2):
                    nc.tensor.matmul(
                        dn, von[:, t, c], pch[:, 2 * c:2 * c + 2, :],
                        start=(c == 0), stop=False, perf_mode=DR)
                nc.tensor.matmul(dn, ones8[:, 0, :],
                                 probs[:, 4 * 256:5 * 256],
                                 start=False, stop=True)
                for c in range(2):
                    tt = t + 2 * c
                    nc.tensor.matmul(
                        otp, vt[:, tt:tt + 2, kv * DV:(kv + 1) * DV],
                        pch[:, 2 * c:2 * c + 2, :],
                        start=(c == 0), stop=False, perf_mode=DR)
                nc.tensor.matmul(
                    otp, vt[:, t + 4, kv * DV:(kv + 1) * DV],
                    probs[:, 4 * 256:5 * 256],
                    start=False, stop=True)
                rec = rec_pool.tile([128, 256], f32)
                nc.vector.reciprocal_approx_fast(rec[:], dn)
                nc.vector.tensor_mul(
                    acat[p][:, t * 256:(t + 1) * 256], otp, rec[:])

            qpair_cur = qT_pool.tile([128, 2 * TOWN], bf, name="qpair")
            qproj_h(0, 0, qpair_cur)
            qproj_h(0, 1, qpair_cur)
            pend = []
            it = 0
            for p in range(H // 2):
                kv = p // 2
                qpair = qpair_cur
                nc.sync.dma_start(emb_res[p][:],
                                  d["emb_own"][p * 128:(p + 1) * 128, :])
                if p + 1 < H // 2:
                    qpair_cur = qT_pool.tile([128, 2 * TOWN], bf, name="qpair")
                    qproj_h(p + 1, 0, qpair_cur)
                for t in range(8):
                    if t == 4 and p + 1 < H // 2:
                        qproj_h(p + 1, 1, qpair_cur)
                    probs = scores_emit(qpair, kv, t, it % 2)
                    pend.append((p, kv, t, probs))
                    if len(pend) > 1:
                        finish_emit(*pend.pop(0))
                    it += 1
            while pend:
                finish_emit(*pend.pop(0))
        emb_cm.__exit__(None, None, None)
        v_cm.__exit__(None, None, None)
        kT_cm.__exit__(None, None, None)

        # ---------------- Stage D: out projection + residual ----------
        with ExitStack() as sd:
            out_pool = sd.enter_context(tc.tile_pool(name="outsb", bufs=3))
            ops_pool = sd.enter_context(
                tc.tile_pool(name="op_ps", bufs=4, space="PSUM"))

            for t in range(8):
                out_sb = out_pool.tile([128, E], bf)
                for j in range(4):
                    op = ops_pool.tile([128, 512], f32)
                    for pr in range(H // 2):
                        lhs = acat[pr][:, t * 256:(t + 1) * 256].rearrange(
                            "p (g q) -> p g q", g=2)
                        nc.tensor.matmul(
                            op[:], lhs,
                            wo8[:, pr, :, j * 512:(j + 1) * 512],
                            start=(pr == 0), stop=(pr == H // 2 - 1),
                            perf_mode=DR)
                    nc.vector.scalar_tensor_tensor(
                        out_sb[:, j * 512:(j + 1) * 512], op[:], RES_SCALE,
                        emb_res[t][:, j * 512:(j + 1) * 512], ALU.mult,
                        ALU.add)
                nc.sync.dma_start(d["out"][t * 128:(t + 1) * 128, :],
                                  out_sb[:])
        acat_cm.__exit__(None, None, None)
        res_cm.__exit__(None, None, None)
        wo_cm.__exit__(None, None, None)


_CACHED_NC = None


def build_graph():
    global _CACHED_NC
    if _CACHED_NC is not None:
        return _CACHED_NC
    nc = bacc.Bacc("TRN2", target_bir_lowering=False, debug=False,
                   enable_asserts=False, num_devices=8)
    d = {}
    d["emb8"] = nc.dram_tensor("emb8", [128, NE, TALL], f8,
                               kind="ExternalInput").ap()
    d["emb_own"] = nc.dram_tensor("emb_own", [TOWN, E], bf,
                                  kind="ExternalInput").ap()
    d["wq"] = nc.dram_tensor("wq", [H, 128, NEP, 2, DK], f8,
                             kind="ExternalInput").ap()
    d["wk"] = nc.dram_tensor("wk", [128, NEP, 2, HKV * DK], f8,
                             kind="ExternalInput").ap()
    d["wv"] = nc.dram_tensor("wv", [128, NEP, 2, HKV * DV], f8,
                             kind="ExternalInput").ap()
    d["wo"] = nc.dram_tensor("wo", [128, H // 2, 2, E], f8,
                             kind="ExternalInput").ap()
    d["cosqT"] = nc.dram_tensor("cosqT", [DK, TOWN], bf, kind="ExternalInput").ap()
    d["sinqT"] = nc.dram_tensor("sinqT", [DK, TOWN], bf, kind="ExternalInput").ap()
    d["coskT"] = nc.dram_tensor("coskT", [DK, TALL], bf, kind="ExternalInput").ap()
    d["sinkT"] = nc.dram_tensor("sinkT", [DK, TALL], bf, kind="ExternalInput").ap()
    d["maskB"] = nc.dram_tensor("maskB", [128, 2, 2, 128], f8,
                                kind="ExternalInput").ap()
    d["vones"] = nc.dram_tensor("vones", [128, NB * NQC, 2, 2, 128], f8,
                                kind="ExternalInput").ap()
    d["out"] = nc.dram_tensor("out", [TOWN, E], bf, kind="ExternalOutput").ap()

    with tile.TileContext(nc, trace_sim=False) as tc:
        build(tc, d)
    nc.compile()
    _CACHED_NC = nc
    return nc


def make_in_maps(embeddings, cos_buffer, sin_buffer, wq, wk, wv, wo):
    embeddings = np.asarray(embeddings, dtype=np.float32)
    cos_buffer = np.asarray(cos_buffer, dtype=np.float32)
    sin_buffer = np.asarray(sin_buffer, dtype=np.float32)

    # fp8 weight packs with power-of-2 scales; DoubleRow pair layouts
    wq_s = (np.asarray(wq, np.float32) * (SQ_W / math.sqrt(DK))).astype(F8)
    # [E, H*DK] -> per head [E, DK] -> [NEP, 2, 128, DK] -> [128, NEP, 2, DK]
    wq_p = np.ascontiguousarray(
        wq_s.reshape(NEP, 2, 128, H, DK).transpose(3, 2, 0, 1, 4))
    wk_p = np.ascontiguousarray(
        (np.asarray(wk, np.float32) * SK_W).astype(F8)
        .reshape(NEP, 2, 128, HKV * DK).transpose(2, 0, 1, 3))
    wv_p = np.ascontiguousarray(
        (np.asarray(wv, np.float32) * SV_W).astype(F8)
        .reshape(NEP, 2, 128, HKV * DV).transpose(2, 0, 1, 3))
    wo_p = np.ascontiguousarray(
        (np.asarray(wo, np.float32) * SO_W).astype(F8)
        .reshape(H // 2, 2, 128, E).transpose(2, 0, 1, 3))

    qq = np.arange(128)
    kk = np.arange(128)
    in_maps = []
    for core in range(8):
        b, c = divmod(core, 4)
        tok0 = 1024 * c
        if c == 0:
            pad = np.zeros((HALO, E), np.float32)
            seg = np.concatenate([pad, embeddings[b, :TOWN]], axis=0)
            padc = np.zeros((HALO, DK), np.float32)
            ck = np.concatenate([padc, cos_buffer[1, 0, :TOWN]], axis=0)
            sk = np.concatenate([padc, sin_buffer[1, 0, :TOWN]], axis=0)
        else:
            seg = embeddings[b, tok0 - HALO:tok0 + TOWN]
            ck = cos_buffer[1, 0, tok0 - HALO:tok0 + TOWN]
            sk = sin_buffer[1, 0, tok0 - HALO:tok0 + TOWN]

        # emb8: [E, TALL] -> [NE, 128, TALL] -> [128, NE, TALL] fp8
        emb8 = np.ascontiguousarray(
            seg.T.astype(F8).reshape(NE, 128, TALL).transpose(1, 0, 2))

        # vones: dn stationary with padded keys zeroed
        # [t=blk*4+qc, pair c, 128(kk), i, 128(out)] -> [128, t, c, i, 128]
        von = np.zeros((NB * NQC, 2, 128, 2, 128), np.float32)
        for blk in range(NB):
            for qc in range(NQC):
                t = 4 * blk + qc
                for c in range(2):
                    for i in range(2):
                        ch = 2 * c + i
                        kpos = (tok0 - 512 + 512 * blk + 128 * qc
                                + 128 * ch + kk)
                        von[t, c, :, i, :] = (kpos >= 0)[:, None]
        vones = np.ascontiguousarray(von.transpose(2, 0, 1, 3, 4))
        # maskB: shared ch0/ch4 window triangles (kk vs qq offsets)
        maskB = np.zeros((128, 2, 2, 128), np.float32)
        mch0 = (kk[:, None] - 512) > (qq[None, :] - WIN)   # j>i strict upper
        mch4 = kk[:, None] <= qq[None, :]                  # lower incl diag
        for g in range(2):
            maskB[:, 0, g, :] = mch0
            maskB[:, 1, g, :] = mch4

        in_maps.append({
            "emb8": emb8,
            "emb_own": np.ascontiguousarray(
                embeddings[b, tok0:tok0 + TOWN]).astype(BF16),
            "wq": wq_p, "wk": wk_p, "wv": wv_p, "wo": wo_p,
            "cosqT": np.ascontiguousarray(
                cos_buffer[0, 0, tok0:tok0 + TOWN].T).astype(BF16),
            "sinqT": np.ascontiguousarray(
                sin_buffer[0, 0, tok0:tok0 + TOWN].T).astype(BF16),
            "coskT": np.ascontiguousarray(ck.T).astype(BF16),
            "sinkT": np.ascontiguousarray(sk.T).astype(BF16),
            "maskB": maskB.astype(F8),
            "vones": vones.astype(F8),
        })
    return in_maps


def _install_ntff_hook():
    """Recreate the missing antenv.axon_hooks registry so
    run_bass_kernel_spmd(trace=True) can capture an NTFF profile."""
    import types
    if "antenv.axon_hooks" not in sys.modules:
        m = types.ModuleType("antenv.axon_hooks")
        m._hook = None
        m.set_axon_ntff_profile_hook = lambda h: setattr(m, "_hook", h)
        m.get_axon_ntff_profile_hook = lambda: m._hook
        sys.modules["antenv.axon_hooks"] = m
        try:
            import antenv
            antenv.axon_hooks = m
        except ImportError:
            pass
    try:
        from trn_agent_boot.trn_boot import _ntff_profile_via_ctypes
        hook = _ntff_profile_via_ctypes("/opt/axon/libaxon_pjrt.so")
        sys.modules["antenv.axon_hooks"].set_axon_ntff_profile_hook(hook)
    except Exception as exc:  # degrade to no tracing
        print(f"ntff hook install failed: {exc}", file=sys.stderr)


def kernel(embeddings, cos_buffer, sin_buffer, wq, wk, wv, wo, window_size,
           trace=False):
    assert int(window_size) == WIN
    if trace:
        _install_ntff_hook()
    nc = build_graph()
    in_maps = make_in_maps(embeddings, cos_buffer, sin_buffer, wq, wk, wv, wo)
    res = bass_utils.run_bass_kernel_spmd(
        nc, in_maps, core_ids=list(range(8)), trace=trace)
    out = np.zeros((B, S, E), np.float32)
    for core in range(8):
        b, c = divmod(core, 4)
        out[b, 1024 * c:1024 * (c + 1)] = np.asarray(
            res.results[core]["out"]).astype(np.float32)
    if trace:
        kernel.last_exec_time_ns = res.exec_time_ns
    return out


kernel.last_exec_time_ns = None
